# revision 12
# baseline (speedup 1.0000x reference)
"""MLA (multi-head latent attention) Trainium2 kernel, 8-core SPMD.

Sharding: data-parallel over batch (B=2) x tensor-parallel over head
groups (16 heads -> 4 per core).  Core c handles batch c//4, heads
4*(c%4) .. 4*(c%4)+3.  Each core computes its partial out = ctx @ W_o
row-slice; the host sums the 4 partials per batch.  c_kv / k_r outputs
are produced per-core (transposed layouts) and reassembled on host.

On-chip layouts are "transposed" (feature dim on partitions):
  qq[:, h, s]: rows 0:64 = q_c/8, rows 64:128 = rope(q_r/8) (perm: evens;odds)
  kk[:, h, s]: rows 0:64 = k_c,   rows 64:128 = rope(k_r)
  scoresT[sk, sq] = sum_d kk[d, sk] * qq[d, sq]  (one K=128 matmul)
  softmax over sk (partitions) without max subtraction (scores are O(8));
  sum(exp) obtained via an extra ones-column in v; the 1/Z row is
  broadcast across partitions with a rank-1 PE matmul.
"""

import math
import os
import sys
from contextlib import ExitStack

import numpy as np

for _p in ("/opt/trn_rl_repo", os.path.expanduser("~/.axon_site/_ro/trn_rl_repo")):
    if os.path.isdir(_p) and _p not in sys.path:
        sys.path.insert(0, _p)

import concourse.bass as bass  # noqa: E402
import concourse.mybir as mybir  # noqa: E402
import concourse.tile as tile  # noqa: E402
from concourse import bacc  # noqa: E402
from concourse.masks import make_identity  # noqa: E402

F32 = mybir.dt.float32
F32R = mybir.dt.float32r

B = 2
D = 1024
NH = 16
DH = 64
DL = 256
DR = 64
HL = 4          # heads per core
NCORES = 8
ST = 512        # s-tile width


def build_nc(S=2048, causal=True, use_mask=False, debug=False):
    NST = S // ST          # sq tiles
    NSB = S // 128         # sk blocks
    KC = D // 128          # contraction chunks over D

    nc = bacc.Bacc("TRN2", target_bir_lowering=False, num_devices=NCORES)

    xb = nc.dram_tensor("xb", [S, D], F32, kind="ExternalInput")
    wq_d = nc.dram_tensor("wq", [128, KC, HL * DH], F32R, kind="ExternalInput")
    wqp_d = nc.dram_tensor("wqp", [128, KC, HL * DR], F32R, kind="ExternalInput")
    wkp_d = nc.dram_tensor("wkp", [128, KC, HL * DR], F32R, kind="ExternalInput")
    wdkv_d = nc.dram_tensor("wdkv", [128, KC, DL], F32R, kind="ExternalInput")
    wuk_d = nc.dram_tensor("wuk", [128, 2, HL * DH], F32R, kind="ExternalInput")
    wuv_d = nc.dram_tensor("wuv", [128, 2, HL * DH], F32R, kind="ExternalInput")
    wo_d = nc.dram_tensor("wo", [128, 2, D], F32R, kind="ExternalInput")
    cos2_d = nc.dram_tensor("cos2", [64, S], F32, kind="ExternalInput")
    sin2_d = nc.dram_tensor("sin2", [64, S], F32, kind="ExternalInput")
    vones_d = nc.dram_tensor("vones", [128, NSB, HL * 65], F32R, kind="ExternalInput")
    ones64_d = nc.dram_tensor("ones64", [1, 64], F32R, kind="ExternalInput")
    if causal:
        tri_d = nc.dram_tensor("tri", [128, 128], F32, kind="ExternalInput")
    if use_mask:
        maskT_d = nc.dram_tensor("maskT", [S, S], F32, kind="ExternalInput")

    outp_d = nc.dram_tensor("outp", [S, D], F32, kind="ExternalOutput")
    ckvT_d = nc.dram_tensor("ckvT", [2, 128, S], F32R, kind="ExternalOutput")
    krT_d = nc.dram_tensor("krT", [HL, 64, S], F32R, kind="ExternalOutput")
    if debug:
        qq_dbg = nc.dram_tensor("qq_dbg", [128, HL, S], F32R, kind="ExternalOutput")
        kk_dbg = nc.dram_tensor("kk_dbg", [128, HL, S], F32R, kind="ExternalOutput")
        vv_dbg = nc.dram_tensor("vv_dbg", [128, NSB, HL * 65], F32R, kind="ExternalOutput")
        ctxT_dbg = nc.dram_tensor("ctxT_dbg", [128, 2, S], F32R, kind="ExternalOutput")

    with tile.TileContext(nc) as tc, ExitStack() as top:
        persist = top.enter_context(tc.tile_pool(name="persist", bufs=1))
        qq = persist.tile([128, HL, S], F32R)
        kk = persist.tile([128, HL, S], F32R)
        vv = persist.tile([128, NSB, HL * 65], F32R)
        ctxT = persist.tile([128, 2, S], F32R)

        # ones everywhere first: the v-copies overwrite all but the per-head
        # 65th column, which stays 1 and yields Z = sum(exp) in the ctx matmul
        nc.sync.dma_start(out=vv, in_=vones_d[:, :, :])

        # ---------------- phase 1: projections ----------------
        with ExitStack() as p1:
            wts = p1.enter_context(tc.tile_pool(name="wts", bufs=1))
            w_q = wts.tile([128, KC, HL * DH], F32R)
            w_qp = wts.tile([128, KC, HL * DR], F32R)
            w_kp = wts.tile([128, KC, HL * DR], F32R)
            w_dkv = wts.tile([128, KC, DL], F32R)
            w_uk = wts.tile([128, 2, HL * DH], F32R)
            w_uv = wts.tile([128, 2, HL * DH], F32R)
            nc.sync.dma_start(out=w_q, in_=wq_d[:, :, :])
            nc.sync.dma_start(out=w_qp, in_=wqp_d[:, :, :])
            nc.sync.dma_start(out=w_kp, in_=wkp_d[:, :, :])
            nc.sync.dma_start(out=w_dkv, in_=wdkv_d[:, :, :])
            nc.sync.dma_start(out=w_uk, in_=wuk_d[:, :, :])
            nc.sync.dma_start(out=w_uv, in_=wuv_d[:, :, :])

            trig = p1.enter_context(tc.tile_pool(name="trig", bufs=1))
            cos2 = trig.tile([64, S], F32)
            sin2 = trig.tile([64, S], F32)
            ident = trig.tile([128, 128], F32)
            nc.sync.dma_start(out=cos2, in_=cos2_d[:, :])
            nc.sync.dma_start(out=sin2, in_=sin2_d[:, :])
            make_identity(nc, ident)

            ckvp = p1.enter_context(tc.tile_pool(name="ckvp", bufs=1))
            ckvT = ckvp.tile([128, 2, S], F32R)

            xnat = p1.enter_context(tc.tile_pool(name="xnat", bufs=3))
            xtp = p1.enter_context(tc.tile_pool(name="xtp", bufs=1))
            ropet = p1.enter_context(tc.tile_pool(name="ropet", bufs=1))
            ps_t = p1.enter_context(tc.tile_pool(name="ps_t", bufs=2, space="PSUM"))
            ps_p = p1.enter_context(tc.tile_pool(name="ps_p", bufs=3, space="PSUM"))
            ps_v = p1.enter_context(tc.tile_pool(name="ps_v", bufs=2, space="PSUM"))

            for st in range(NST):
                s0 = st * ST
                xT = xtp.tile([128, KC, ST], F32R, tag="xT")
                # transpose x[s0:s0+ST, :] -> xT (d on partitions)
                for ss in range(ST // 128):
                    xn = xnat.tile([128, D], F32, tag="xn")
                    nc.sync.dma_start(out=xn, in_=xb[s0 + ss * 128 : s0 + (ss + 1) * 128, :])
                    for c2 in range(KC // 4):
                        pt = ps_t.tile([128, 4, 128], F32, tag="pt")
                        for j in range(4):
                            c = 4 * c2 + j
                            nc.tensor.transpose(
                                pt[:, j, :], xn[:, c * 128 : (c + 1) * 128], ident
                            )
                        nc.scalar.copy(
                            out=xT[:, 4 * c2 : 4 * c2 + 4, ss * 128 : (ss + 1) * 128],
                            in_=pt,
                        )

                # q_c (scaled 1/8 on host) -> qq rows 0:64
                for n in range(2):
                    ps = ps_p.tile([128, ST], F32, tag="proj")
                    for c in range(KC):
                        nc.tensor.matmul(
                            ps, w_q[:, c, 128 * n : 128 * n + 128], xT[:, c, :],
                            start=(c == 0), stop=(c == KC - 1),
                        )
                    nc.scalar.copy(out=qq[0:64, 2 * n, s0 : s0 + ST], in_=ps[0:64, :])
                    nc.scalar.copy(out=qq[0:64, 2 * n + 1, s0 : s0 + ST], in_=ps[64:128, :])

                # c_kv -> ckvT
                for n in range(2):
                    ps = ps_p.tile([128, ST], F32, tag="proj")
                    for c in range(KC):
                        nc.tensor.matmul(
                            ps, w_dkv[:, c, 128 * n : 128 * n + 128], xT[:, c, :],
                            start=(c == 0), stop=(c == KC - 1),
                        )
                    nc.scalar.copy(out=ckvT[:, n, s0 : s0 + ST], in_=ps)

                # roped projections -> qq/kk rows 64:128
                for w_t, dst in ((w_qp, qq), (w_kp, kk)):
                    for n in range(2):
                        ps = ps_p.tile([128, ST], F32, tag="proj")
                        for c in range(KC):
                            nc.tensor.matmul(
                                ps, w_t[:, c, 128 * n : 128 * n + 128], xT[:, c, :],
                                start=(c == 0), stop=(c == KC - 1),
                            )
                        ta = ropet.tile([64, ST], F32, tag="ta")
                        tb = ropet.tile([64, ST], F32, tag="tb")
                        tcs = ropet.tile([64, ST], F32, tag="tc")
                        td = ropet.tile([64, ST], F32, tag="td")
                        co = cos2[:, s0 : s0 + ST]
                        si = sin2[:, s0 : s0 + ST]
                        nc.vector.tensor_mul(ta, ps[0:64, :], co)
                        nc.vector.tensor_mul(tb, ps[64:128, :], si)
                        nc.vector.tensor_mul(tcs, ps[0:64, :], si)
                        nc.vector.tensor_mul(td, ps[64:128, :], co)
                        ha, hb = 2 * n, 2 * n + 1
                        sl = slice(s0, s0 + ST)
                        nc.vector.tensor_sub(dst[64:96, ha, sl], ta[0:32, :], tb[0:32, :])
                        nc.vector.tensor_sub(dst[64:96, hb, sl], ta[32:64, :], tb[32:64, :])
                        nc.vector.tensor_add(dst[96:128, ha, sl], tcs[0:32, :], td[0:32, :])
                        nc.vector.tensor_add(dst[96:128, hb, sl], tcs[32:64, :], td[32:64, :])

                # k_c = W_uk^T @ c_kv^T -> kk rows 0:64
                for n in range(2):
                    ps = ps_p.tile([128, ST], F32, tag="proj")
                    for c in range(2):
                        nc.tensor.matmul(
                            ps, w_uk[:, c, 128 * n : 128 * n + 128],
                            ckvT[:, c, s0 : s0 + ST],
                            start=(c == 0), stop=(c == 1),
                        )
                    nc.scalar.copy(out=kk[0:64, 2 * n, s0 : s0 + ST], in_=ps[0:64, :])
                    nc.scalar.copy(out=kk[0:64, 2 * n + 1, s0 : s0 + ST], in_=ps[64:128, :])

                # v_c natural [sk, d] (+ ones col already set)
                for sb in range(ST // 128):
                    blk = st * (ST // 128) + sb
                    pv = ps_v.tile([128, HL * DH], F32, tag="pv")
                    for c in range(2):
                        nc.tensor.matmul(
                            pv, ckvT[:, c, s0 + sb * 128 : s0 + (sb + 1) * 128],
                            w_uv[:, c, :],
                            start=(c == 0), stop=(c == 1),
                        )
                    for h in range(HL):
                        nc.vector.tensor_copy(
                            out=vv[:, blk, 65 * h : 65 * h + 64],
                            in_=pv[:, 64 * h : 64 * h + 64],
                        )

            # stream outputs of phase 1
            for c in range(2):
                nc.sync.dma_start(out=ckvT_d[c, :, :], in_=ckvT[:, c, :])
            for h in range(HL):
                nc.sync.dma_start(out=krT_d[h, :, :], in_=kk[64:128, h, :])

        # ---------------- phase 2+3: attention + out ----------------
        with ExitStack() as p2:
            misc = p2.enter_context(tc.tile_pool(name="misc", bufs=1))
            w_o = misc.tile([128, 2, D], F32R)
            nc.sync.dma_start(out=w_o, in_=wo_d[:, :, :])
            ones64 = misc.tile([1, 64], F32R)
            nc.sync.dma_start(out=ones64, in_=ones64_d[:, :])
            if causal:
                tri_sb = misc.tile([128, 128], F32)
                nc.sync.dma_start(out=tri_sb, in_=tri_d[:, :])

            wp = p2.enter_context(tc.tile_pool(name="wp", bufs=4))
            rp = p2.enter_context(tc.tile_pool(name="rp", bufs=2))
            obp = p2.enter_context(tc.tile_pool(name="obp", bufs=2))
            if use_mask:
                mp = p2.enter_context(tc.tile_pool(name="mp", bufs=3))
            ps_s = p2.enter_context(tc.tile_pool(name="ps_s", bufs=3, space="PSUM"))
            ps_c = p2.enter_context(tc.tile_pool(name="ps_c", bufs=2, space="PSUM"))
            ps_r = p2.enter_context(tc.tile_pool(name="ps_r", bufs=1, space="PSUM"))
            ps_o = p2.enter_context(tc.tile_pool(name="ps_o", bufs=2, space="PSUM"))

            for jt in range(NST):
                q0 = jt * ST
                nblk = 4 * (jt + 1) if causal else NSB
                for h in range(HL):
                    pc = ps_c.tile([65, ST], F32, tag="ctx")
                    for i in range(nblk):
                        off = max(0, 128 * i - q0) if causal else 0
                        ps = ps_s.tile([128, ST], F32, tag="sc")
                        nc.tensor.matmul(
                            ps[:, off:ST], kk[:, h, 128 * i : 128 * i + 128],
                            qq[:, h, q0 + off : q0 + ST],
                            start=True, stop=True,
                        )
                        if use_mask:
                            mt = mp.tile([128, ST], F32, tag="mt")
                            nc.sync.dma_start(
                                out=mt, in_=maskT_d[128 * i : 128 * i + 128, q0 : q0 + ST]
                            )
                            nc.vector.tensor_add(ps, ps, mt)
                        if causal and i >= 4 * jt:
                            nc.vector.tensor_add(
                                ps[:, off : off + 128], ps[:, off : off + 128], tri_sb
                            )
                        w = wp.tile([128, ST], F32R, tag="w")
                        nc.scalar.activation(
                            out=w[:, off:ST], in_=ps[:, off:ST],
                            func=mybir.ActivationFunctionType.Exp,
                        )
                        # i==0 always has off==0, so the first matmul of the
                        # accumulation group covers the full [65, ST] region.
                        nc.tensor.matmul(
                            pc[:, off:ST], vv[:, i, 65 * h : 65 * h + 65], w[:, off:ST],
                            start=(i == 0), stop=(i == nblk - 1),
                        )
                    # normalize: ctxT[:, h] = pc[0:64] / Z,  Z = pc[64]
                    rr = rp.tile([1, ST], F32R, tag="rr")
                    with nc.allow_low_precision("fp32r reciprocal is plenty here"):
                        nc.vector.reciprocal(rr, pc[64:65, :])
                    pr = ps_r.tile([64, ST], F32, tag="pr")
                    nc.tensor.matmul(pr, ones64, rr, start=True, stop=True)
                    rb = rp.tile([64, ST], F32, tag="rb")
                    nc.scalar.copy(out=rb, in_=pr)
                    nc.vector.tensor_mul(
                        ctxT[64 * (h % 2) : 64 * (h % 2) + 64, h // 2, q0 : q0 + ST],
                        pc[0:64, :], rb,
                    )
                # out projection for this sq tile
                for sb in range(ST // 128):
                    r0 = q0 + sb * 128
                    ob = obp.tile([128, D], F32, tag="ob")
                    for mtile in range(2):
                        po = ps_o.tile([128, ST], F32, tag="po")
                        for c in range(2):
                            nc.tensor.matmul(
                                po, ctxT[:, c, r0 : r0 + 128],
                                w_o[:, c, mtile * ST : (mtile + 1) * ST],
                                start=(c == 0), stop=(c == 1),
                            )
                        nc.vector.tensor_copy(out=ob[:, mtile * ST : (mtile + 1) * ST], in_=po)
                    nc.sync.dma_start(out=outp_d[r0 : r0 + 128, :], in_=ob)

            if debug:
                nc.sync.dma_start(out=qq_dbg[:, :, :], in_=qq)
                nc.sync.dma_start(out=kk_dbg[:, :, :], in_=kk)
                nc.sync.dma_start(out=vv_dbg[:, :, :], in_=vv)
                nc.sync.dma_start(out=ctxT_dbg[:, :, :], in_=ctxT)

    nc.finalize()
    return nc


# ---------------------------------------------------------------------------
# host side
# ---------------------------------------------------------------------------

def _rope_tables_np(S):
    theta = 1.0 / (10000.0 ** (np.arange(0, DR, 2, dtype=np.float32) / DR))
    freqs = np.outer(np.arange(S, dtype=np.float32), theta)  # [S, 32]
    return np.cos(freqs).T.copy(), np.sin(freqs).T.copy()    # [32, S]


def _chunk(w, kc):
    """[kc*128, N] -> [128, kc, N] contiguous"""
    n = w.shape[1]
    return np.ascontiguousarray(
        w.reshape(kc, 128, n).transpose(1, 0, 2), dtype=np.float32
    )


def _perm_cols(g):
    """column order for W_{q,k}_pos slice of head group g (len 256)"""
    cols = []
    for n in range(2):
        ha, hb = 4 * g + 2 * n, 4 * g + 2 * n + 1
        cols += [64 * ha + 2 * i for i in range(32)]       # ha evens
        cols += [64 * hb + 2 * i for i in range(32)]       # hb evens
        cols += [64 * ha + 2 * i + 1 for i in range(32)]   # ha odds
        cols += [64 * hb + 2 * i + 1 for i in range(32)]   # hb odds
    return np.array(cols)


def prep_in_maps(x, attn_mask, W_q, W_dkv, W_uk, W_uv, W_k_pos, W_q_pos, W_o, S):
    KC = D // 128
    cosT, sinT = _rope_tables_np(S)
    cos2 = np.concatenate([cosT, cosT], 0)
    sin2 = np.concatenate([sinT, sinT], 0)

    m = np.asarray(attn_mask, np.float32).reshape(S, S)
    causal_ref = np.where(np.tril(np.ones((S, S), bool)), 0.0, -1e9).astype(np.float32)
    if np.array_equal(m, causal_ref):
        causal, use_mask = True, False
    elif not m.any():
        causal, use_mask = False, False
    else:
        causal, use_mask = False, True

    tri = np.where(
        np.arange(128)[:, None] <= np.arange(128)[None, :], 0.0, -1e9
    ).astype(np.float32)

    in_maps = []
    for core in range(NCORES):
        b, g = core // 4, core % 4
        sl = slice(256 * g, 256 * g + 256)
        pc = _perm_cols(g)
        im = {
            "xb": np.ascontiguousarray(x[b], np.float32),
            "wq": _chunk(W_q[:, sl] * 0.125, KC),
            "wqp": _chunk(W_q_pos[:, pc] * 0.125, KC),
            "wkp": _chunk(W_k_pos[:, pc], KC),
            "wdkv": _chunk(W_dkv, KC),
            "wuk": _chunk(W_uk[:, sl], 2),
            "wuv": _chunk(W_uv[:, sl], 2),
            "wo": _chunk(W_o[sl, :], 2),
            "cos2": cos2,
            "sin2": sin2,
            "vones": np.ones((128, (S // 128) * HL * 65), np.float32).reshape(
                128, S // 128, HL * 65
            ),
            "ones64": np.ones((1, 64), np.float32),
        }
        if causal:
            im["tri"] = tri
        if use_mask:
            im["maskT"] = np.ascontiguousarray(m.T)
        in_maps.append(im)
    return in_maps, causal, use_mask


def assemble(results, S):
    out = np.zeros((B, S, D), np.float32)
    c_kv = np.zeros((B, S, DL), np.float32)
    k_r = np.zeros((B, NH, S, DR), np.float32)
    for core in range(NCORES):
        b, g = core // 4, core % 4
        r = results[core]
        out[b] += r["outp"]
        if g == 0:
            c_kv[b] = r["ckvT"].reshape(DL, S).T
        krT = r["krT"]  # [HL, 64, S]
        for h in range(HL):
            k_r[b, 4 * g + h, :, 0::2] = krT[h, 0:32, :].T
            k_r[b, 4 * g + h, :, 1::2] = krT[h, 32:64, :].T
    return out, c_kv, k_r


_NC_CACHE = {}


def get_nc(S, causal, use_mask):
    key = (S, causal, use_mask)
    if key not in _NC_CACHE:
        _NC_CACHE[key] = build_nc(S, causal, use_mask)
    return _NC_CACHE[key]


def kernel(x, attn_mask, W_q, W_dkv, W_uk, W_uv, W_k_pos, W_q_pos, W_o,
           _trace=False, _trace_kwargs=None):
    from concourse.bass_utils import run_bass_kernel_spmd

    x = np.asarray(x, np.float32)
    S = x.shape[1]
    args = [np.asarray(a, np.float32) for a in
            (W_q, W_dkv, W_uk, W_uv, W_k_pos, W_q_pos, W_o)]
    in_maps, causal, use_mask = prep_in_maps(x, attn_mask, args[0], args[1],
                                             args[2], args[3], args[4], args[5],
                                             args[6], S)
    nc = get_nc(S, causal, use_mask)
    res = run_bass_kernel_spmd(
        nc, in_maps, list(range(NCORES)),
        trace=_trace, **(_trace_kwargs or {}),
    )
    out = assemble(res.results, S)
    if _trace:
        return out, res
    return out


# revision 14
# speedup vs baseline: 1.0178x; 1.0178x over previous
"""MLA (multi-head latent attention) Trainium2 kernel, 8-core SPMD.

Sharding: data-parallel over batch (B=2) x tensor-parallel over head
groups (16 heads -> 4 per core).  Core c handles batch c//4, heads
4*(c%4) .. 4*(c%4)+3.  Each core computes its partial out = ctx @ W_o
row-slice; the host sums the 4 partials per batch.  c_kv / k_r outputs
are produced per-core (transposed layouts) and reassembled on host.

On-chip layouts are "transposed" (feature dim on partitions):
  qq[:, h, s]: rows 0:64 = q_c/8, rows 64:128 = rope(q_r/8) (perm: evens;odds)
  kk[:, h, s]: rows 0:64 = k_c,   rows 64:128 = rope(k_r)
  scoresT[sk, sq] = sum_d kk[d, sk] * qq[d, sq]  (one K=128 matmul)
  softmax over sk (partitions) without max subtraction (scores are O(8));
  sum(exp) obtained via an extra ones-column in v; the 1/Z row is
  broadcast across partitions with a rank-1 PE matmul.
"""

import math
import os
import sys
from contextlib import ExitStack

import numpy as np

for _p in ("/opt/trn_rl_repo", os.path.expanduser("~/.axon_site/_ro/trn_rl_repo")):
    if os.path.isdir(_p) and _p not in sys.path:
        sys.path.insert(0, _p)

import concourse.bass as bass  # noqa: E402
import concourse.mybir as mybir  # noqa: E402
import concourse.tile as tile  # noqa: E402
from concourse import bacc  # noqa: E402
from concourse.masks import make_identity  # noqa: E402

F32 = mybir.dt.float32
F32R = mybir.dt.float32r

B = 2
D = 1024
NH = 16
DH = 64
DL = 256
DR = 64
HL = 4          # heads per core
NCORES = 8
ST = 512        # s-tile width


def build_nc(S=2048, causal=True, use_mask=False, debug=False):
    NST = S // ST          # sq tiles
    NSB = S // 128         # sk blocks
    KC = D // 128          # contraction chunks over D

    nc = bacc.Bacc("TRN2", target_bir_lowering=False, num_devices=NCORES)

    xb = nc.dram_tensor("xb", [S, D], F32R, kind="ExternalInput")
    wq_d = nc.dram_tensor("wq", [128, KC, HL * DH], F32R, kind="ExternalInput")
    wqp_d = nc.dram_tensor("wqp", [128, KC, HL * DR], F32R, kind="ExternalInput")
    wkp_d = nc.dram_tensor("wkp", [128, KC, HL * DR], F32R, kind="ExternalInput")
    wdkv_d = nc.dram_tensor("wdkv", [128, KC, DL], F32R, kind="ExternalInput")
    wuk_d = nc.dram_tensor("wuk", [128, 2, HL * DH], F32R, kind="ExternalInput")
    wuv_d = nc.dram_tensor("wuv", [128, 2, HL * DH], F32R, kind="ExternalInput")
    wo_d = nc.dram_tensor("wo", [128, 2, D], F32R, kind="ExternalInput")
    cos2_d = nc.dram_tensor("cos2", [64, S], F32, kind="ExternalInput")
    sin2_d = nc.dram_tensor("sin2", [64, S], F32, kind="ExternalInput")
    vones_d = nc.dram_tensor("vones", [128, HL * 65], F32R, kind="ExternalInput")
    eye_d = nc.dram_tensor("eye", [128, 128], F32R, kind="ExternalInput")
    ones64_d = nc.dram_tensor("ones64", [1, 64], F32R, kind="ExternalInput")
    if causal:
        tri_d = nc.dram_tensor("tri", [128, 128], F32, kind="ExternalInput")
    if use_mask:
        maskT_d = nc.dram_tensor("maskT", [S, S], F32, kind="ExternalInput")

    outp_d = nc.dram_tensor("outp", [S, D], F32, kind="ExternalOutput")
    ckvT_d = nc.dram_tensor("ckvT", [2, 128, S], F32R, kind="ExternalOutput")
    krT_d = nc.dram_tensor("krT", [HL, 64, S], F32R, kind="ExternalOutput")
    if debug:
        qq_dbg = nc.dram_tensor("qq_dbg", [128, HL, S], F32R, kind="ExternalOutput")
        kk_dbg = nc.dram_tensor("kk_dbg", [128, HL, S], F32R, kind="ExternalOutput")
        vv_dbg = nc.dram_tensor("vv_dbg", [128, NSB, HL * 65], F32R, kind="ExternalOutput")
        ctxT_dbg = nc.dram_tensor("ctxT_dbg", [128, 2, S], F32R, kind="ExternalOutput")

    with tile.TileContext(nc) as tc, ExitStack() as top:
        persist = top.enter_context(tc.tile_pool(name="persist", bufs=1))
        qq = persist.tile([128, HL, S], F32R)
        kk = persist.tile([128, HL, S], F32R)
        vv = persist.tile([128, NSB, HL * 65], F32R)
        ctxT = persist.tile([128, 2, S], F32R)

        # ones everywhere first: the v-copies overwrite all but the per-head
        # 65th column, which stays 1 and yields Z = sum(exp) in the ctx matmul
        vones_bc = bass.AP(tensor=vones_d, offset=0,
                           ap=[[HL * 65, 128], [0, NSB], [1, HL * 65]])
        nc.gpsimd.dma_start(out=vv, in_=vones_bc)

        # ---------------- phase 1: projections ----------------
        with ExitStack() as p1:
            wts = p1.enter_context(tc.tile_pool(name="wts", bufs=1))
            w_q = wts.tile([128, KC, HL * DH], F32R)
            w_qp = wts.tile([128, KC, HL * DR], F32R)
            w_kp = wts.tile([128, KC, HL * DR], F32R)
            w_dkv = wts.tile([128, KC, DL], F32R)
            w_uk = wts.tile([128, 2, HL * DH], F32R)
            w_uv = wts.tile([128, 2, HL * DH], F32R)
            nc.sync.dma_start(out=w_q, in_=wq_d[:, :, :])
            nc.sync.dma_start(out=w_qp, in_=wqp_d[:, :, :])
            nc.sync.dma_start(out=w_kp, in_=wkp_d[:, :, :])
            nc.sync.dma_start(out=w_dkv, in_=wdkv_d[:, :, :])
            nc.sync.dma_start(out=w_uk, in_=wuk_d[:, :, :])
            nc.sync.dma_start(out=w_uv, in_=wuv_d[:, :, :])

            trig = p1.enter_context(tc.tile_pool(name="trig", bufs=1))
            cos2 = trig.tile([64, S], F32)
            sin2 = trig.tile([64, S], F32)
            ident = trig.tile([128, 128], F32R)
            nc.sync.dma_start(out=cos2, in_=cos2_d[:, :])
            nc.sync.dma_start(out=sin2, in_=sin2_d[:, :])
            nc.sync.dma_start(out=ident, in_=eye_d[:, :])

            ckvp = p1.enter_context(tc.tile_pool(name="ckvp", bufs=1))
            ckvT = ckvp.tile([128, 2, S], F32R)

            xnat = p1.enter_context(tc.tile_pool(name="xnat", bufs=3))
            xtp = p1.enter_context(tc.tile_pool(name="xtp", bufs=1))
            ropet = p1.enter_context(tc.tile_pool(name="ropet", bufs=1))
            ps_t = p1.enter_context(tc.tile_pool(name="ps_t", bufs=2, space="PSUM"))
            ps_p = p1.enter_context(tc.tile_pool(name="ps_p", bufs=3, space="PSUM"))
            ps_v = p1.enter_context(tc.tile_pool(name="ps_v", bufs=2, space="PSUM"))

            for st in range(NST):
                s0 = st * ST
                xT = xtp.tile([128, KC, ST], F32R, tag="xT")
                # transpose x[s0:s0+ST, :] -> xT (d on partitions)
                for ss in range(ST // 128):
                    xn = xnat.tile([128, D], F32R, tag="xn")
                    nc.scalar.dma_start(out=xn, in_=xb[s0 + ss * 128 : s0 + (ss + 1) * 128, :])
                    for c2 in range(KC // 4):
                        pt = ps_t.tile([128, 4, 128], F32R, tag="pt")
                        for j in range(4):
                            c = 4 * c2 + j
                            nc.tensor.transpose(
                                pt[:, j, :], xn[:, c * 128 : (c + 1) * 128], ident
                            )
                        nc.scalar.copy(
                            out=xT[:, 4 * c2 : 4 * c2 + 4, ss * 128 : (ss + 1) * 128],
                            in_=pt,
                        )

                # q_c (scaled 1/8 on host) -> qq rows 0:64
                for n in range(2):
                    ps = ps_p.tile([128, ST], F32, tag="proj")
                    for c in range(KC):
                        nc.tensor.matmul(
                            ps, w_q[:, c, 128 * n : 128 * n + 128], xT[:, c, :],
                            start=(c == 0), stop=(c == KC - 1),
                        )
                    nc.scalar.copy(out=qq[0:64, 2 * n, s0 : s0 + ST], in_=ps[0:64, :])
                    nc.scalar.copy(out=qq[0:64, 2 * n + 1, s0 : s0 + ST], in_=ps[64:128, :])

                # c_kv -> ckvT
                for n in range(2):
                    ps = ps_p.tile([128, ST], F32, tag="proj")
                    for c in range(KC):
                        nc.tensor.matmul(
                            ps, w_dkv[:, c, 128 * n : 128 * n + 128], xT[:, c, :],
                            start=(c == 0), stop=(c == KC - 1),
                        )
                    nc.scalar.copy(out=ckvT[:, n, s0 : s0 + ST], in_=ps)

                # roped projections -> qq/kk rows 64:128
                for w_t, dst in ((w_qp, qq), (w_kp, kk)):
                    for n in range(2):
                        ps = ps_p.tile([128, ST], F32, tag="proj")
                        for c in range(KC):
                            nc.tensor.matmul(
                                ps, w_t[:, c, 128 * n : 128 * n + 128], xT[:, c, :],
                                start=(c == 0), stop=(c == KC - 1),
                            )
                        ta = ropet.tile([64, ST], F32, tag="ta")
                        tb = ropet.tile([64, ST], F32, tag="tb")
                        tcs = ropet.tile([64, ST], F32, tag="tc")
                        td = ropet.tile([64, ST], F32, tag="td")
                        co = cos2[:, s0 : s0 + ST]
                        si = sin2[:, s0 : s0 + ST]
                        nc.vector.tensor_mul(ta, ps[0:64, :], co)
                        nc.vector.tensor_mul(tb, ps[64:128, :], si)
                        nc.vector.tensor_mul(tcs, ps[0:64, :], si)
                        nc.vector.tensor_mul(td, ps[64:128, :], co)
                        ha, hb = 2 * n, 2 * n + 1
                        sl = slice(s0, s0 + ST)
                        nc.vector.tensor_sub(dst[64:96, ha, sl], ta[0:32, :], tb[0:32, :])
                        nc.vector.tensor_sub(dst[64:96, hb, sl], ta[32:64, :], tb[32:64, :])
                        nc.vector.tensor_add(dst[96:128, ha, sl], tcs[0:32, :], td[0:32, :])
                        nc.vector.tensor_add(dst[96:128, hb, sl], tcs[32:64, :], td[32:64, :])

                # k_c = W_uk^T @ c_kv^T -> kk rows 0:64
                for n in range(2):
                    ps = ps_p.tile([128, ST], F32, tag="proj")
                    for c in range(2):
                        nc.tensor.matmul(
                            ps, w_uk[:, c, 128 * n : 128 * n + 128],
                            ckvT[:, c, s0 : s0 + ST],
                            start=(c == 0), stop=(c == 1),
                        )
                    nc.scalar.copy(out=kk[0:64, 2 * n, s0 : s0 + ST], in_=ps[0:64, :])
                    nc.scalar.copy(out=kk[0:64, 2 * n + 1, s0 : s0 + ST], in_=ps[64:128, :])

                # v_c natural [sk, d] (+ ones col already set)
                for sb in range(ST // 128):
                    blk = st * (ST // 128) + sb
                    pv = ps_v.tile([128, HL * DH], F32, tag="pv")
                    for c in range(2):
                        nc.tensor.matmul(
                            pv, ckvT[:, c, s0 + sb * 128 : s0 + (sb + 1) * 128],
                            w_uv[:, c, :],
                            start=(c == 0), stop=(c == 1),
                        )
                    for h in range(HL):
                        nc.scalar.copy(
                            out=vv[:, blk, 65 * h : 65 * h + 64],
                            in_=pv[:, 64 * h : 64 * h + 64],
                        )

            # stream outputs of phase 1
            for c in range(2):
                nc.gpsimd.dma_start(out=ckvT_d[c, :, :], in_=ckvT[:, c, :])
            for h in range(HL):
                nc.gpsimd.dma_start(out=krT_d[h, :, :], in_=kk[64:128, h, :])

        # ---------------- phase 2+3: attention + out ----------------
        with ExitStack() as p2:
            misc = p2.enter_context(tc.tile_pool(name="misc", bufs=1))
            w_o = misc.tile([128, 2, D], F32R)
            nc.sync.dma_start(out=w_o, in_=wo_d[:, :, :])
            ones64 = misc.tile([1, 64], F32R)
            nc.sync.dma_start(out=ones64, in_=ones64_d[:, :])
            if causal:
                tri_sb = misc.tile([128, 128], F32)
                nc.sync.dma_start(out=tri_sb, in_=tri_d[:, :])

            wp = p2.enter_context(tc.tile_pool(name="wp", bufs=4))
            rp = p2.enter_context(tc.tile_pool(name="rp", bufs=2))
            obp = p2.enter_context(tc.tile_pool(name="obp", bufs=2))
            if use_mask:
                mp = p2.enter_context(tc.tile_pool(name="mp", bufs=3))
            ps_s = p2.enter_context(tc.tile_pool(name="ps_s", bufs=3, space="PSUM"))
            ps_c = p2.enter_context(tc.tile_pool(name="ps_c", bufs=2, space="PSUM"))
            ps_r = p2.enter_context(tc.tile_pool(name="ps_r", bufs=1, space="PSUM"))
            ps_o = p2.enter_context(tc.tile_pool(name="ps_o", bufs=2, space="PSUM"))

            for jt in range(NST):
                q0 = jt * ST
                nblk = 4 * (jt + 1) if causal else NSB
                for h in range(HL):
                    pc = ps_c.tile([65, ST], F32, tag="ctx")
                    for i in range(nblk):
                        off = max(0, 128 * i - q0) if causal else 0
                        ps = ps_s.tile([128, ST], F32, tag="sc")
                        nc.tensor.matmul(
                            ps[:, off:ST], kk[:, h, 128 * i : 128 * i + 128],
                            qq[:, h, q0 + off : q0 + ST],
                            start=True, stop=True,
                        )
                        if use_mask:
                            mt = mp.tile([128, ST], F32, tag="mt")
                            nc.sync.dma_start(
                                out=mt, in_=maskT_d[128 * i : 128 * i + 128, q0 : q0 + ST]
                            )
                            nc.vector.tensor_add(ps, ps, mt)
                        if causal and i >= 4 * jt:
                            nc.vector.tensor_add(
                                ps[:, off : off + 128], ps[:, off : off + 128], tri_sb
                            )
                        w = wp.tile([128, ST], F32R, tag="w")
                        nc.scalar.activation(
                            out=w[:, off:ST], in_=ps[:, off:ST],
                            func=mybir.ActivationFunctionType.Exp,
                        )
                        # i==0 always has off==0, so the first matmul of the
                        # accumulation group covers the full [65, ST] region.
                        nc.tensor.matmul(
                            pc[:, off:ST], vv[:, i, 65 * h : 65 * h + 65], w[:, off:ST],
                            start=(i == 0), stop=(i == nblk - 1),
                        )
                    # normalize: ctxT[:, h] = pc[0:64] / Z,  Z = pc[64]
                    rr = rp.tile([1, ST], F32R, tag="rr")
                    with nc.allow_low_precision("fp32r reciprocal is plenty here"):
                        nc.vector.reciprocal(rr, pc[64:65, :])
                    pr = ps_r.tile([64, ST], F32, tag="pr")
                    nc.tensor.matmul(pr, ones64, rr, start=True, stop=True)
                    rb = rp.tile([64, ST], F32, tag="rb")
                    nc.vector.tensor_copy(out=rb, in_=pr)
                    nc.vector.tensor_mul(
                        ctxT[64 * (h % 2) : 64 * (h % 2) + 64, h // 2, q0 : q0 + ST],
                        pc[0:64, :], rb,
                    )
                # out projection for this sq tile
                for sb in range(ST // 128):
                    r0 = q0 + sb * 128
                    ob = obp.tile([128, D], F32, tag="ob")
                    for mtile in range(2):
                        po = ps_o.tile([128, ST], F32, tag="po")
                        for c in range(2):
                            nc.tensor.matmul(
                                po, ctxT[:, c, r0 : r0 + 128],
                                w_o[:, c, mtile * ST : (mtile + 1) * ST],
                                start=(c == 0), stop=(c == 1),
                            )
                        nc.vector.tensor_copy(out=ob[:, mtile * ST : (mtile + 1) * ST], in_=po)
                    nc.gpsimd.dma_start(out=outp_d[r0 : r0 + 128, :], in_=ob)

            if debug:
                nc.sync.dma_start(out=qq_dbg[:, :, :], in_=qq)
                nc.sync.dma_start(out=kk_dbg[:, :, :], in_=kk)
                nc.sync.dma_start(out=vv_dbg[:, :, :], in_=vv)
                nc.sync.dma_start(out=ctxT_dbg[:, :, :], in_=ctxT)

    nc.finalize()
    return nc


# ---------------------------------------------------------------------------
# host side
# ---------------------------------------------------------------------------

def _rope_tables_np(S):
    theta = 1.0 / (10000.0 ** (np.arange(0, DR, 2, dtype=np.float32) / DR))
    freqs = np.outer(np.arange(S, dtype=np.float32), theta)  # [S, 32]
    return np.cos(freqs).T.copy(), np.sin(freqs).T.copy()    # [32, S]


def _chunk(w, kc):
    """[kc*128, N] -> [128, kc, N] contiguous"""
    n = w.shape[1]
    return np.ascontiguousarray(
        w.reshape(kc, 128, n).transpose(1, 0, 2), dtype=np.float32
    )


def _perm_cols(g):
    """column order for W_{q,k}_pos slice of head group g (len 256)"""
    cols = []
    for n in range(2):
        ha, hb = 4 * g + 2 * n, 4 * g + 2 * n + 1
        cols += [64 * ha + 2 * i for i in range(32)]       # ha evens
        cols += [64 * hb + 2 * i for i in range(32)]       # hb evens
        cols += [64 * ha + 2 * i + 1 for i in range(32)]   # ha odds
        cols += [64 * hb + 2 * i + 1 for i in range(32)]   # hb odds
    return np.array(cols)


def prep_in_maps(x, attn_mask, W_q, W_dkv, W_uk, W_uv, W_k_pos, W_q_pos, W_o, S):
    KC = D // 128
    cosT, sinT = _rope_tables_np(S)
    cos2 = np.concatenate([cosT, cosT], 0)
    sin2 = np.concatenate([sinT, sinT], 0)

    m = np.asarray(attn_mask, np.float32).reshape(S, S)
    causal_ref = np.where(np.tril(np.ones((S, S), bool)), 0.0, -1e9).astype(np.float32)
    if np.array_equal(m, causal_ref):
        causal, use_mask = True, False
    elif not m.any():
        causal, use_mask = False, False
    else:
        causal, use_mask = False, True

    tri = np.where(
        np.arange(128)[:, None] <= np.arange(128)[None, :], 0.0, -1e9
    ).astype(np.float32)

    in_maps = []
    for core in range(NCORES):
        b, g = core // 4, core % 4
        sl = slice(256 * g, 256 * g + 256)
        pc = _perm_cols(g)
        im = {
            "xb": np.ascontiguousarray(x[b], np.float32),
            "wq": _chunk(W_q[:, sl] * 0.125, KC),
            "wqp": _chunk(W_q_pos[:, pc] * 0.125, KC),
            "wkp": _chunk(W_k_pos[:, pc], KC),
            "wdkv": _chunk(W_dkv, KC),
            "wuk": _chunk(W_uk[:, sl], 2),
            "wuv": _chunk(W_uv[:, sl], 2),
            "wo": _chunk(W_o[sl, :], 2),
            "cos2": cos2,
            "sin2": sin2,
            "vones": np.ones((128, HL * 65), np.float32),
            "ones64": np.ones((1, 64), np.float32),
            "eye": np.eye(128, dtype=np.float32),
        }
        if causal:
            im["tri"] = tri
        if use_mask:
            im["maskT"] = np.ascontiguousarray(m.T)
        in_maps.append(im)
    return in_maps, causal, use_mask


def assemble(results, S):
    out = np.zeros((B, S, D), np.float32)
    c_kv = np.zeros((B, S, DL), np.float32)
    k_r = np.zeros((B, NH, S, DR), np.float32)
    for core in range(NCORES):
        b, g = core // 4, core % 4
        r = results[core]
        out[b] += r["outp"]
        if g == 0:
            c_kv[b] = r["ckvT"].reshape(DL, S).T
        krT = r["krT"]  # [HL, 64, S]
        for h in range(HL):
            k_r[b, 4 * g + h, :, 0::2] = krT[h, 0:32, :].T
            k_r[b, 4 * g + h, :, 1::2] = krT[h, 32:64, :].T
    return out, c_kv, k_r


_NC_CACHE = {}


def get_nc(S, causal, use_mask):
    key = (S, causal, use_mask)
    if key not in _NC_CACHE:
        _NC_CACHE[key] = build_nc(S, causal, use_mask)
    return _NC_CACHE[key]


def kernel(x, attn_mask, W_q, W_dkv, W_uk, W_uv, W_k_pos, W_q_pos, W_o,
           _trace=False, _trace_kwargs=None):
    from concourse.bass_utils import run_bass_kernel_spmd

    x = np.asarray(x, np.float32)
    S = x.shape[1]
    args = [np.asarray(a, np.float32) for a in
            (W_q, W_dkv, W_uk, W_uv, W_k_pos, W_q_pos, W_o)]
    in_maps, causal, use_mask = prep_in_maps(x, attn_mask, args[0], args[1],
                                             args[2], args[3], args[4], args[5],
                                             args[6], S)
    nc = get_nc(S, causal, use_mask)
    res = run_bass_kernel_spmd(
        nc, in_maps, list(range(NCORES)),
        trace=_trace, **(_trace_kwargs or {}),
    )
    out = assemble(res.results, S)
    if _trace:
        return out, res
    return out


# revision 32
# speedup vs baseline: 1.3065x; 1.2836x over previous
"""MLA (multi-head latent attention) Trainium2 kernel, 8-core SPMD.

Sharding: data-parallel over batch (B=2) x tensor-parallel over head
groups (16 heads -> 4 per core).  Core c handles batch c//4, heads
4*(c%4) .. 4*(c%4)+3.  Each core computes its partial out = ctx @ W_o
row-slice; the host sums the 4 partials per batch.  c_kv / k_r outputs
are produced per-core (transposed layouts) and reassembled on host.

On-chip layouts are "transposed" (feature dim on partitions):
  qq[:, h, s]: rows 0:64 = q_c/8, rows 64:128 = rope(q_r/8) (perm: evens;odds)
  kk[:, h, s]: rows 0:64 = k_c,   rows 64:128 = rope(k_r)
  scoresT[sk, sq] = sum_d kk[d, sk] * qq[d, sq]  (one K=128 matmul)
  softmax over sk (partitions) without max subtraction (scores are O(8));
  sum(exp) obtained via an extra ones-column in v; the 1/Z row is
  broadcast across partitions with a rank-1 PE matmul.
"""

import math
import os
import sys
from contextlib import ExitStack

import numpy as np

for _p in ("/opt/trn_rl_repo", os.path.expanduser("~/.axon_site/_ro/trn_rl_repo")):
    if os.path.isdir(_p) and _p not in sys.path:
        sys.path.insert(0, _p)

import concourse.bass as bass  # noqa: E402
import concourse.mybir as mybir  # noqa: E402
import concourse.tile as tile  # noqa: E402
from concourse import bacc  # noqa: E402
from concourse.masks import make_identity  # noqa: E402

F32 = mybir.dt.float32
F32R = mybir.dt.float32r

B = 2
D = 1024
NH = 16
DH = 64
DL = 256
DR = 64
HL = 4          # heads per core
NCORES = 8
ST = 512        # s-tile width


def build_nc(S=2048, causal=True, use_mask=False, debug=False):
    NST = S // ST          # sq tiles
    NSB = S // 128         # sk blocks
    KC = D // 128          # contraction chunks over D

    nc = bacc.Bacc("TRN2", target_bir_lowering=False, num_devices=NCORES)

    xbT_d = nc.dram_tensor("xbT", [KC, 128, S], F32R, kind="ExternalInput")
    wq_d = nc.dram_tensor("wq", [128, KC, HL * DH], F32R, kind="ExternalInput")
    wqp_d = nc.dram_tensor("wqp", [128, KC, HL * DR], F32R, kind="ExternalInput")
    wkp_d = nc.dram_tensor("wkp", [128, KC, HL * DR], F32R, kind="ExternalInput")
    wdkv_d = nc.dram_tensor("wdkv", [128, KC, DL], F32R, kind="ExternalInput")
    wuk_d = nc.dram_tensor("wuk", [128, 2, HL * DH], F32R, kind="ExternalInput")
    wuv_d = nc.dram_tensor("wuv", [128, 2, HL * DH], F32R, kind="ExternalInput")
    wo_d = nc.dram_tensor("wo", [128, 2, D], F32R, kind="ExternalInput")
    cos2_d = nc.dram_tensor("cos2", [64, S], F32, kind="ExternalInput")
    sin2_d = nc.dram_tensor("sin2", [64, S], F32, kind="ExternalInput")
    vones_d = nc.dram_tensor("vones", [128, HL * 65], F32R, kind="ExternalInput")
    ones64_d = nc.dram_tensor("ones64", [1, 64], F32R, kind="ExternalInput")
    if causal:
        tri_d = nc.dram_tensor("tri", [128, 128], F32, kind="ExternalInput")
    if use_mask:
        maskT_d = nc.dram_tensor("maskT", [S, S], F32, kind="ExternalInput")

    outp_d = nc.dram_tensor("outp", [S, D], F32, kind="ExternalOutput")
    ckvT_d = nc.dram_tensor("ckvT", [2, 128, S], F32R, kind="ExternalOutput")
    krT_d = nc.dram_tensor("krT", [HL, 64, S], F32R, kind="ExternalOutput")
    if debug:
        qq_dbg = nc.dram_tensor("qq_dbg", [128, HL, S], F32R, kind="ExternalOutput")
        kk_dbg = nc.dram_tensor("kk_dbg", [128, HL, S], F32R, kind="ExternalOutput")
        vv_dbg = nc.dram_tensor("vv_dbg", [128, NSB, HL * 65], F32R, kind="ExternalOutput")
        ctxT_dbg = nc.dram_tensor("ctxT_dbg", [128, 2, S], F32R, kind="ExternalOutput")

    with tile.TileContext(nc) as tc, ExitStack() as top:
        persist = top.enter_context(tc.tile_pool(name="persist", bufs=1))
        qq = persist.tile([128, HL, S], F32R)
        kk = persist.tile([128, HL, S], F32R)
        vv = persist.tile([128, NSB, HL * 65], F32R)
        ctxT = persist.tile([128, 2, S], F32R)

        # ones everywhere first: the v-copies overwrite all but the per-head
        # 65th column, which stays 1 and yields Z = sum(exp) in the ctx matmul
        vones_bc = bass.AP(tensor=vones_d, offset=0,
                           ap=[[HL * 65, 128], [0, NSB], [1, HL * 65]])
        nc.gpsimd.dma_start(out=vv, in_=vones_bc)

        wts = top.enter_context(tc.tile_pool(name="wts", bufs=1))
        w_q = wts.tile([128, KC, HL * DH], F32R)
        w_qp = wts.tile([128, KC, HL * DR], F32R)
        w_kp = wts.tile([128, KC, HL * DR], F32R)
        w_dkv = wts.tile([128, KC, DL], F32R)
        w_uk = wts.tile([128, 2, HL * DH], F32R)
        w_uv = wts.tile([128, 2, HL * DH], F32R)
        w_o = wts.tile([128, 2, D], F32R)
        ones64 = wts.tile([1, 64], F32R)
        if causal:
            tri_sb = wts.tile([128, 128], F32)

        def weight_chunks():
            cs_ = []

            def ld(t, d):
                nc.sync.dma_start(out=t, in_=d[:, :, :])
            cs_.append(lambda: (ld(w_q, wq_d), ld(w_dkv, wdkv_d)))
            cs_.append(lambda: (ld(w_qp, wqp_d), ld(w_kp, wkp_d)))
            cs_.append(lambda: (ld(w_uk, wuk_d), ld(w_uv, wuv_d)))

            def ld_misc():
                nc.sync.dma_start(out=ones64, in_=ones64_d[:, :])
                if causal:
                    nc.sync.dma_start(out=tri_sb, in_=tri_d[:, :])
            cs_.append(ld_misc)
            cs_.append(lambda: ld(w_o, wo_d))
            return cs_

        trig = top.enter_context(tc.tile_pool(name="trig", bufs=1))
        ckvp = top.enter_context(tc.tile_pool(name="ckvp", bufs=2))
        xtp = top.enter_context(tc.tile_pool(name="xtp", bufs=1))
        ropet = top.enter_context(tc.tile_pool(name="ropet", bufs=1))
        wp = top.enter_context(tc.tile_pool(name="wp", bufs=3))
        rp = top.enter_context(tc.tile_pool(name="rp", bufs=1))
        obp = top.enter_context(tc.tile_pool(name="obp", bufs=1))
        if use_mask:
            mp = top.enter_context(tc.tile_pool(name="mp", bufs=3))
        # PSUM: 8 banks total, shared by tag across uses
        ps_a = top.enter_context(tc.tile_pool(name="ps_a", bufs=2, space="PSUM"))
        ps_b = top.enter_context(tc.tile_pool(name="ps_b", bufs=2, space="PSUM"))
        ps_ctx = top.enter_context(tc.tile_pool(name="ps_ctx", bufs=2, space="PSUM"))

        def proj_chunks(st):
            chunks = []
            ck = chunks.append
            s0 = st * ST
            ckvT = ckvp.tile([128, 2, ST], F32R, tag="ckvT")
            cs = trig.tile([128, ST], F32, tag="cs")   # [cos; sin]
            sc = trig.tile([128, ST], F32, tag="sc")   # [sin; cos]
            nc.sync.dma_start(out=cs[0:64, :], in_=cos2_d[:, s0 : s0 + ST])
            nc.sync.dma_start(out=cs[64:128, :], in_=sin2_d[:, s0 : s0 + ST])
            nc.sync.dma_start(out=sc[0:64, :], in_=sin2_d[:, s0 : s0 + ST])
            nc.sync.dma_start(out=sc[64:128, :], in_=cos2_d[:, s0 : s0 + ST])
            xT = xtp.tile([128, KC, ST], F32R, tag="xT")

            def load_xT(c):
                nc.gpsimd.dma_start(out=xT[:, c, :], in_=xbT_d[c, :, s0 : s0 + ST])
            for c in range(KC):
                ck(lambda c=c: load_xT(c))

            # q_c (pre-scaled 1/8) -> qq rows 0:64
            def qc_chunk(n):
                ps = ps_b.tile([128, ST], F32, tag="b")
                for c in range(KC):
                    nc.tensor.matmul(
                        ps, w_q[:, c, 128 * n : 128 * n + 128], xT[:, c, :],
                        start=(c == 0), stop=(c == KC - 1))
                nc.scalar.copy(out=qq[0:64, 2 * n, s0 : s0 + ST], in_=ps[0:64, :])
                nc.vector.tensor_copy(out=qq[0:64, 2 * n + 1, s0 : s0 + ST], in_=ps[64:128, :])
            for n in range(2):
                ck(lambda n=n: qc_chunk(n))

            # c_kv -> ckvT (streamed out; feeds k_c / v_c below)
            def ckv_chunk(n):
                ps = ps_b.tile([128, ST], F32, tag="b")
                for c in range(KC):
                    nc.tensor.matmul(
                        ps, w_dkv[:, c, 128 * n : 128 * n + 128], xT[:, c, :],
                        start=(c == 0), stop=(c == KC - 1))
                nc.scalar.copy(out=ckvT[:, n, :], in_=ps)
            for n in range(2):
                ck(lambda n=n: ckv_chunk(n))

            def ckv_dma():
                ckv_out = bass.AP(tensor=ckvT_d, offset=s0,
                                  ap=[[S, 128], [128 * S, 2], [1, ST]])
                nc.gpsimd.dma_start(out=ckv_out, in_=ckvT)
            ck(ckv_dma)

            # roped projections -> qq/kk rows 64:128
            def rope_chunk(w_t, dst, n):
                ps = ps_b.tile([128, ST], F32, tag="b")
                for c in range(KC):
                    nc.tensor.matmul(
                        ps, w_t[:, c, 128 * n : 128 * n + 128], xT[:, c, :],
                        start=(c == 0), stop=(c == KC - 1))
                ta = ropet.tile([64, ST], F32, tag="ta")   # x1*cos
                tb = ropet.tile([64, ST], F32, tag="tb")   # x2*sin
                tc_ = ropet.tile([64, ST], F32, tag="tc")  # x1*sin
                td = ropet.tile([64, ST], F32, tag="td")   # x2*cos
                nc.vector.tensor_mul(ta, ps[0:64, :], cs[0:64, :])
                nc.vector.tensor_mul(tb, ps[64:128, :], cs[64:128, :])
                nc.vector.tensor_mul(tc_, ps[0:64, :], sc[0:64, :])
                nc.vector.tensor_mul(td, ps[64:128, :], sc[64:128, :])
                ha, hb = 2 * n, 2 * n + 1
                sl = slice(s0, s0 + ST)
                nc.gpsimd.tensor_sub(dst[64:96, ha, sl], ta[0:32, :], tb[0:32, :])
                nc.gpsimd.tensor_sub(dst[64:96, hb, sl], ta[32:64, :], tb[32:64, :])
                nc.gpsimd.tensor_add(dst[96:128, ha, sl], tc_[0:32, :], td[0:32, :])
                nc.gpsimd.tensor_add(dst[96:128, hb, sl], tc_[32:64, :], td[32:64, :])
            for w_t, dst in ((w_qp, qq), (w_kp, kk)):
                for n in range(2):
                    ck(lambda w_t=w_t, dst=dst, n=n: rope_chunk(w_t, dst, n))

            # k_c = W_uk^T @ c_kv^T -> kk rows 0:64
            def kc_chunk(n):
                ps = ps_b.tile([128, ST], F32, tag="b")
                for c in range(2):
                    nc.tensor.matmul(
                        ps, w_uk[:, c, 128 * n : 128 * n + 128], ckvT[:, c, :],
                        start=(c == 0), stop=(c == 1))
                nc.scalar.copy(out=kk[0:64, 2 * n, s0 : s0 + ST], in_=ps[0:64, :])
                nc.scalar.copy(out=kk[0:64, 2 * n + 1, s0 : s0 + ST], in_=ps[64:128, :])
            for n in range(2):
                ck(lambda n=n: kc_chunk(n))

            # v_c natural [sk, d]
            def v_chunk(sb):
                blk = st * (ST // 128) + sb
                pv = ps_b.tile([128, HL * DH], F32, tag="b")
                for c in range(2):
                    nc.tensor.matmul(
                        pv, ckvT[:, c, sb * 128 : (sb + 1) * 128], w_uv[:, c, :],
                        start=(c == 0), stop=(c == 1))
                for h in range(HL):
                    if h < 1:
                        nc.scalar.copy(
                            out=vv[:, blk, 65 * h : 65 * h + 64],
                            in_=pv[:, 64 * h : 64 * h + 64])
                    else:
                        nc.vector.tensor_copy(
                            out=vv[:, blk, 65 * h : 65 * h + 64],
                            in_=pv[:, 64 * h : 64 * h + 64])
            for sb in range(ST // 128):
                ck(lambda sb=sb: v_chunk(sb))

            # stream k_r out as each tile of kk rows 64:128 completes
            def kr_dma():
                for h in range(HL):
                    nc.gpsimd.dma_start(
                        out=krT_d[h, :, s0 : s0 + ST], in_=kk[64:128, h, s0 : s0 + ST])
            ck(kr_dma)
            return chunks

        def attn_chunks(jt):
            chunks = []
            ck = chunks.append
            q0 = jt * ST
            nblk = 4 * (jt + 1) if causal else NSB
            def do_norm(h, pc):
                # normalize: ctxT[:, h] = pc[0:64] / Z,  Z = pc[64]
                rr = rp.tile([1, ST], F32, tag="rr")
                with nc.allow_low_precision("fp32 reciprocal"):
                    nc.vector.reciprocal(rr, pc[64:65, :])
                rb = rp.tile([64, ST], F32, tag="rb")
                nc.gpsimd.partition_broadcast(rb, rr[0:1, :])
                nc.vector.tensor_mul(
                    ctxT[64 * (h % 2) : 64 * (h % 2) + 64, h // 2, q0 : q0 + ST],
                    pc[0:64, :], rb)

            LAG = 3
            state = {"pcs": {}, "pend": [], "norm_q": []}

            def emit_ctx(h, flush_last):
                i0, off0, w0, base = state["pend"].pop(0)
                pc = state["pcs"][h]
                nc.tensor.matmul(
                    pc[:, off0:ST], vv[:, i0, 65 * h : 65 * h + 65],
                    w0[:, base + off0 : base + ST],
                    start=(i0 == 0), stop=(flush_last and i0 == nblk - 1))

            def blk_chunk(h, i, two):
                # processes sk-blocks i (and i+1 when two=True, both full)
                if i == 0:
                    pc_new = ps_ctx.tile([65, ST], F32, tag="ctx")
                    state["pcs"][h] = pc_new
                off = max(0, 128 * i - q0) if causal else 0
                width = 2 * ST if two else ST
                ps = ps_a.tile([128, width], F32, tag="a")
                nc.tensor.matmul(
                    ps[:, off:ST], kk[:, h, 128 * i : 128 * i + 128],
                    qq[:, h, q0 + off : q0 + ST],
                    start=True, stop=True)
                if two:
                    # full width even in the diag region: the sq<sk columns are
                    # junk but finite, never consumed, and keep the psum fully
                    # written so one exp can span the pair
                    nc.tensor.matmul(
                        ps[:, ST : 2 * ST],
                        kk[:, h, 128 * (i + 1) : 128 * (i + 2)],
                        qq[:, h, q0 : q0 + ST],
                        start=True, stop=True)
                if use_mask:
                    mt = mp.tile([128, ST], F32, tag="mt")
                    nc.sync.dma_start(
                        out=mt, in_=maskT_d[128 * i : 128 * i + 128, q0 : q0 + ST])
                    nc.vector.tensor_add(ps[:, 0:ST], ps[:, 0:ST], mt)
                    if two:
                        mt2 = mp.tile([128, ST], F32, tag="mt")
                        nc.sync.dma_start(
                            out=mt2,
                            in_=maskT_d[128 * (i + 1) : 128 * (i + 2), q0 : q0 + ST])
                        nc.vector.tensor_add(ps[:, ST : 2 * ST], ps[:, ST : 2 * ST], mt2)
                off2 = (max(0, 128 * (i + 1) - q0) if causal else 0) if two else 0
                if causal and i >= 4 * jt:
                    nc.vector.tensor_add(
                        ps[:, off : off + 128], ps[:, off : off + 128], tri_sb)
                if two and causal and i + 1 >= 4 * jt:
                    nc.vector.tensor_add(
                        ps[:, ST + off2 : ST + off2 + 128],
                        ps[:, ST + off2 : ST + off2 + 128], tri_sb)
                w = wp.tile([128, width], F32R, tag="w")
                nc.scalar.activation(
                    out=w[:, off:width], in_=ps[:, off:width],
                    func=mybir.ActivationFunctionType.Exp)
                # i==0 always has off==0, so the first matmul of the
                # accumulation group covers the full [65, ST] region.
                state["pend"].append((i, off, w, 0))
                if two:
                    state["pend"].append((i + 1, off2, w, ST))
                while len(state["pend"]) > LAG:
                    emit_ctx(h, False)
                if 1 <= i <= 2 and state["norm_q"]:
                    do_norm(*state["norm_q"].pop())

            def head_flush(h):
                while state["pend"]:
                    emit_ctx(h, True)
                state["norm_q"].append((h, state["pcs"][h]))

            for h in range(HL):
                for i in range(0, nblk, 2):
                    ck(lambda h=h, i=i: blk_chunk(h, i, True))
                ck(lambda h=h: head_flush(h))
            ck(lambda: do_norm(*state["norm_q"].pop()))

            # out projection for this sq tile
            def out_chunk(sb):
                r0 = q0 + sb * 128
                ob = obp.tile([128, D], F32, tag="ob")
                for mtile in range(2):
                    po = ps_b.tile([128, ST], F32, tag="b")
                    for c in range(2):
                        nc.tensor.matmul(
                            po, ctxT[:, c, r0 : r0 + 128],
                            w_o[:, c, mtile * ST : (mtile + 1) * ST],
                            start=(c == 0), stop=(c == 1))
                    nc.vector.tensor_copy(
                        out=ob[:, mtile * ST : (mtile + 1) * ST], in_=po)
                nc.gpsimd.dma_start(out=outp_d[r0 : r0 + 128, :], in_=ob)
            for sb in range(ST // 128):
                ck(lambda sb=sb: out_chunk(sb))
            return chunks

        def run_interleaved(A, B):
            ia = ib = 0
            while ia < len(A) or ib < len(B):
                fa = ia / len(A) if A else 2.0
                fb = ib / len(B) if B else 2.0
                if ia < len(A) and (fa <= fb or ib >= len(B)):
                    A[ia]()
                    ia += 1
                else:
                    B[ib]()
                    ib += 1

        if causal:
            # fused: attention for sq-tile st needs only s-tiles <= st;
            # interleave attn(st) with proj(st+1) for engine overlap
            run_interleaved(proj_chunks(0), weight_chunks())
            for st in range(NST):
                A = attn_chunks(st)
                Bc = proj_chunks(st + 1) if st + 1 < NST else []
                run_interleaved(A, Bc)
        else:
            run_interleaved(proj_chunks(0), weight_chunks())
            for st in range(1, NST):
                for c in proj_chunks(st):
                    c()
            for jt in range(NST):
                for c in attn_chunks(jt):
                    c()

        if debug:
            nc.sync.dma_start(out=qq_dbg[:, :, :], in_=qq)
            nc.sync.dma_start(out=kk_dbg[:, :, :], in_=kk)
            nc.sync.dma_start(out=vv_dbg[:, :, :], in_=vv)
            nc.sync.dma_start(out=ctxT_dbg[:, :, :], in_=ctxT)

    nc.finalize()
    return nc


# ---------------------------------------------------------------------------
# host side
# ---------------------------------------------------------------------------

def _rope_tables_np(S):
    theta = 1.0 / (10000.0 ** (np.arange(0, DR, 2, dtype=np.float32) / DR))
    freqs = np.outer(np.arange(S, dtype=np.float32), theta)  # [S, 32]
    return np.cos(freqs).T.copy(), np.sin(freqs).T.copy()    # [32, S]


def _chunk(w, kc):
    """[kc*128, N] -> [128, kc, N] contiguous"""
    n = w.shape[1]
    return np.ascontiguousarray(
        w.reshape(kc, 128, n).transpose(1, 0, 2), dtype=np.float32
    )


def _perm_cols(g):
    """column order for W_{q,k}_pos slice of head group g (len 256)"""
    cols = []
    for n in range(2):
        ha, hb = 4 * g + 2 * n, 4 * g + 2 * n + 1
        cols += [64 * ha + 2 * i for i in range(32)]       # ha evens
        cols += [64 * hb + 2 * i for i in range(32)]       # hb evens
        cols += [64 * ha + 2 * i + 1 for i in range(32)]   # ha odds
        cols += [64 * hb + 2 * i + 1 for i in range(32)]   # hb odds
    return np.array(cols)


def prep_in_maps(x, attn_mask, W_q, W_dkv, W_uk, W_uv, W_k_pos, W_q_pos, W_o, S):
    KC = D // 128
    cosT, sinT = _rope_tables_np(S)
    cos2 = np.concatenate([cosT, cosT], 0)
    sin2 = np.concatenate([sinT, sinT], 0)

    m = np.asarray(attn_mask, np.float32).reshape(S, S)
    causal_ref = np.where(np.tril(np.ones((S, S), bool)), 0.0, -1e9).astype(np.float32)
    if np.array_equal(m, causal_ref):
        causal, use_mask = True, False
    elif not m.any():
        causal, use_mask = False, False
    else:
        causal, use_mask = False, True

    tri = np.where(
        np.arange(128)[:, None] <= np.arange(128)[None, :], 0.0, -1e9
    ).astype(np.float32)

    in_maps = []
    for core in range(NCORES):
        b, g = core // 4, core % 4
        sl = slice(256 * g, 256 * g + 256)
        pc = _perm_cols(g)
        im = {
            "xbT": np.ascontiguousarray(
                x[b].T.reshape(D // 128, 128, S), np.float32),
            "wq": _chunk(W_q[:, sl] * 0.125, KC),
            "wqp": _chunk(W_q_pos[:, pc] * 0.125, KC),
            "wkp": _chunk(W_k_pos[:, pc], KC),
            "wdkv": _chunk(W_dkv, KC),
            "wuk": _chunk(W_uk[:, sl], 2),
            "wuv": _chunk(W_uv[:, sl], 2),
            "wo": _chunk(W_o[sl, :], 2),
            "cos2": cos2,
            "sin2": sin2,
            "vones": np.ones((128, HL * 65), np.float32),
            "ones64": np.ones((1, 64), np.float32),
        }
        if causal:
            im["tri"] = tri
        if use_mask:
            im["maskT"] = np.ascontiguousarray(m.T)
        in_maps.append(im)
    return in_maps, causal, use_mask


def assemble(results, S):
    out = np.zeros((B, S, D), np.float32)
    c_kv = np.zeros((B, S, DL), np.float32)
    k_r = np.zeros((B, NH, S, DR), np.float32)
    for core in range(NCORES):
        b, g = core // 4, core % 4
        r = results[core]
        out[b] += r["outp"]
        if g == 0:
            c_kv[b] = r["ckvT"].reshape(DL, S).T
        krT = r["krT"]  # [HL, 64, S]
        for h in range(HL):
            k_r[b, 4 * g + h, :, 0::2] = krT[h, 0:32, :].T
            k_r[b, 4 * g + h, :, 1::2] = krT[h, 32:64, :].T
    return out, c_kv, k_r


_NC_CACHE = {}


def get_nc(S, causal, use_mask):
    key = (S, causal, use_mask)
    if key not in _NC_CACHE:
        _NC_CACHE[key] = build_nc(S, causal, use_mask)
    return _NC_CACHE[key]


def kernel(x, attn_mask, W_q, W_dkv, W_uk, W_uv, W_k_pos, W_q_pos, W_o,
           _trace=False, _trace_kwargs=None):
    from concourse.bass_utils import run_bass_kernel_spmd

    x = np.asarray(x, np.float32)
    S = x.shape[1]
    args = [np.asarray(a, np.float32) for a in
            (W_q, W_dkv, W_uk, W_uv, W_k_pos, W_q_pos, W_o)]
    in_maps, causal, use_mask = prep_in_maps(x, attn_mask, args[0], args[1],
                                             args[2], args[3], args[4], args[5],
                                             args[6], S)
    nc = get_nc(S, causal, use_mask)
    res = run_bass_kernel_spmd(
        nc, in_maps, list(range(NCORES)),
        trace=_trace, **(_trace_kwargs or {}),
    )
    out = assemble(res.results, S)
    if _trace:
        return out, res
    return out


# revision 35
# speedup vs baseline: 1.3578x; 1.0393x over previous
"""MLA (multi-head latent attention) Trainium2 kernel, 8-core SPMD.

Sharding: data-parallel over batch (B=2) x tensor-parallel over head
groups (16 heads -> 4 per core).  Core c handles batch c//4, heads
4*(c%4) .. 4*(c%4)+3.  Each core computes its partial out = ctx @ W_o
row-slice; the host sums the 4 partials per batch.  c_kv / k_r outputs
are produced per-core (transposed layouts) and reassembled on host.

On-chip layouts are "transposed" (feature dim on partitions):
  qq[:, h, s]: rows 0:64 = q_c/8, rows 64:128 = rope(q_r/8) (perm: evens;odds)
  kk[:, h, s]: rows 0:64 = k_c,   rows 64:128 = rope(k_r)
  scoresT[sk, sq] = sum_d kk[d, sk] * qq[d, sq]  (one K=128 matmul)
  softmax over sk (partitions) without max subtraction (scores are O(8));
  sum(exp) obtained via an extra ones-column in v; the 1/Z row is
  replicated across partitions with gpsimd partition_broadcast.

Pipeline: projections for s-tile st+1 are emitted interleaved with
attention for sq-tile st (causal => attention(st) only needs s-tiles
<= st); within attention, sk-blocks are processed in pairs (one exp per
pair) and the ctx matmuls trail the scores/exp stream by LAG blocks so
the in-order PE never stalls on the activation engine.
"""

import math
import os
import sys
from contextlib import ExitStack

import numpy as np

for _p in ("/opt/trn_rl_repo", os.path.expanduser("~/.axon_site/_ro/trn_rl_repo")):
    if os.path.isdir(_p) and _p not in sys.path:
        sys.path.insert(0, _p)

import concourse.bass as bass  # noqa: E402
import concourse.mybir as mybir  # noqa: E402
import concourse.tile as tile  # noqa: E402
from concourse import bacc  # noqa: E402
from concourse.masks import make_identity  # noqa: E402

F32 = mybir.dt.float32
F32R = mybir.dt.float32r

B = 2
D = 1024
NH = 16
DH = 64
DL = 256
DR = 64
HL = 4          # heads per core
NCORES = 8
ST = 512        # s-tile width


def build_nc(S=2048, causal=True, use_mask=False, debug=False):
    NST = S // ST          # sq tiles
    NSB = S // 128         # sk blocks
    KC = D // 128          # contraction chunks over D

    nc = bacc.Bacc("TRN2", target_bir_lowering=False, num_devices=NCORES)

    xbT_d = nc.dram_tensor("xbT", [KC, 128, S], F32R, kind="ExternalInput")
    wq_d = nc.dram_tensor("wq", [128, KC, HL * DH], F32R, kind="ExternalInput")
    wqp_d = nc.dram_tensor("wqp", [128, KC, HL * DR], F32R, kind="ExternalInput")
    wkp_d = nc.dram_tensor("wkp", [128, KC, HL * DR], F32R, kind="ExternalInput")
    wdkv_d = nc.dram_tensor("wdkv", [128, KC, DL], F32R, kind="ExternalInput")
    wuk_d = nc.dram_tensor("wuk", [128, 2, HL * DH], F32R, kind="ExternalInput")
    wuv_d = nc.dram_tensor("wuv", [128, 2, HL * DH], F32R, kind="ExternalInput")
    wo_d = nc.dram_tensor("wo", [128, 2, D], F32R, kind="ExternalInput")
    cos2_d = nc.dram_tensor("cos2", [64, S], F32, kind="ExternalInput")
    sin2_d = nc.dram_tensor("sin2", [64, S], F32, kind="ExternalInput")
    vones_d = nc.dram_tensor("vones", [128, HL * 65], F32R, kind="ExternalInput")
    ones64_d = nc.dram_tensor("ones64", [1, 64], F32R, kind="ExternalInput")
    if causal:
        tri_d = nc.dram_tensor("tri", [128, 128], F32, kind="ExternalInput")
    if use_mask:
        maskT_d = nc.dram_tensor("maskT", [S, S], F32, kind="ExternalInput")

    outp_d = nc.dram_tensor("outp", [S, D], F32, kind="ExternalOutput")
    ckvT_d = nc.dram_tensor("ckvT", [2, 128, S], F32R, kind="ExternalOutput")
    krT_d = nc.dram_tensor("krT", [HL, 64, S], F32R, kind="ExternalOutput")
    if debug:
        qq_dbg = nc.dram_tensor("qq_dbg", [128, HL, S], F32R, kind="ExternalOutput")
        kk_dbg = nc.dram_tensor("kk_dbg", [128, HL, S], F32R, kind="ExternalOutput")
        vv_dbg = nc.dram_tensor("vv_dbg", [128, NSB, HL * 65], F32R, kind="ExternalOutput")
        ctxT_dbg = nc.dram_tensor("ctxT_dbg", [128, 2, S], F32R, kind="ExternalOutput")

    with tile.TileContext(nc) as tc, ExitStack() as top:
        persist = top.enter_context(tc.tile_pool(name="persist", bufs=1))
        qq = persist.tile([128, HL, S], F32R)
        kk = persist.tile([128, HL, S], F32R)
        vv = persist.tile([128, NSB, HL * 65], F32R)
        ctxT = persist.tile([128, 2, S], F32R)

        # ones everywhere first: the v-copies overwrite all but the per-head
        # 65th column, which stays 1 and yields Z = sum(exp) in the ctx matmul
        vones_bc = bass.AP(tensor=vones_d, offset=0,
                           ap=[[HL * 65, 128], [0, NSB], [1, HL * 65]])
        nc.gpsimd.dma_start(out=vv, in_=vones_bc)

        wts = top.enter_context(tc.tile_pool(name="wts", bufs=1))
        w_q = wts.tile([128, KC, HL * DH], F32R)
        w_qp = wts.tile([128, KC, HL * DR], F32R)
        w_kp = wts.tile([128, KC, HL * DR], F32R)
        w_dkv = wts.tile([128, KC, DL], F32R)
        w_uk = wts.tile([128, 2, HL * DH], F32R)
        w_uv = wts.tile([128, 2, HL * DH], F32R)
        w_o = wts.tile([128, 2, D], F32R)
        ones64 = wts.tile([1, 64], F32R)
        if causal:
            tri_sb = wts.tile([128, 128], F32)

        def weight_chunks():
            cs_ = []

            def ld(t, d):
                nc.sync.dma_start(out=t, in_=d[:, :, :])
            cs_.append(lambda: (ld(w_q, wq_d), ld(w_dkv, wdkv_d)))
            cs_.append(lambda: (ld(w_qp, wqp_d), ld(w_kp, wkp_d)))
            cs_.append(lambda: (ld(w_uk, wuk_d), ld(w_uv, wuv_d)))

            def ld_misc():
                nc.sync.dma_start(out=ones64, in_=ones64_d[:, :])
                if causal:
                    nc.sync.dma_start(out=tri_sb, in_=tri_d[:, :])
            cs_.append(ld_misc)
            cs_.append(lambda: ld(w_o, wo_d))
            return cs_

        trig = top.enter_context(tc.tile_pool(name="trig", bufs=1))
        ckvp = top.enter_context(tc.tile_pool(name="ckvp", bufs=2))
        xtp = top.enter_context(tc.tile_pool(name="xtp", bufs=1))
        ropet = top.enter_context(tc.tile_pool(name="ropet", bufs=1))
        wp = top.enter_context(tc.tile_pool(name="wp", bufs=5))
        rp = top.enter_context(tc.tile_pool(name="rp", bufs=1))
        obp = top.enter_context(tc.tile_pool(name="obp", bufs=1))
        if use_mask:
            mp = top.enter_context(tc.tile_pool(name="mp", bufs=3))
        # PSUM: 8 banks total, shared by tag across uses
        ps_a = top.enter_context(tc.tile_pool(name="ps_a", bufs=2, space="PSUM"))
        ps_b = top.enter_context(tc.tile_pool(name="ps_b", bufs=2, space="PSUM"))
        ps_ctx = top.enter_context(tc.tile_pool(name="ps_ctx", bufs=2, space="PSUM"))

        def proj_chunks(st):
            chunks = []
            ck = chunks.append
            s0 = st * ST
            ckvT = ckvp.tile([128, 2, ST], F32R, tag="ckvT")
            cs = trig.tile([128, ST], F32, tag="cs")   # [cos; sin]
            sc = trig.tile([128, ST], F32, tag="sc")   # [sin; cos]
            nc.sync.dma_start(out=cs[0:64, :], in_=cos2_d[:, s0 : s0 + ST])
            nc.sync.dma_start(out=cs[64:128, :], in_=sin2_d[:, s0 : s0 + ST])
            nc.sync.dma_start(out=sc[0:64, :], in_=sin2_d[:, s0 : s0 + ST])
            nc.sync.dma_start(out=sc[64:128, :], in_=cos2_d[:, s0 : s0 + ST])
            xT = xtp.tile([128, KC, ST], F32R, tag="xT")

            def load_xT(c):
                nc.gpsimd.dma_start(out=xT[:, c, :], in_=xbT_d[c, :, s0 : s0 + ST])
            for c in range(KC):
                ck(lambda c=c: load_xT(c))

            # q_c (pre-scaled 1/8) -> qq rows 0:64
            def qc_chunk(n):
                ps = ps_b.tile([128, ST], F32, tag="b")
                for c in range(KC):
                    nc.tensor.matmul(
                        ps, w_q[:, c, 128 * n : 128 * n + 128], xT[:, c, :],
                        start=(c == 0), stop=(c == KC - 1))
                nc.scalar.copy(out=qq[0:64, 2 * n, s0 : s0 + ST], in_=ps[0:64, :])
                nc.vector.tensor_copy(out=qq[0:64, 2 * n + 1, s0 : s0 + ST], in_=ps[64:128, :])
            for n in range(2):
                ck(lambda n=n: qc_chunk(n))

            # c_kv -> ckvT (streamed out; feeds k_c / v_c below)
            def ckv_chunk(n):
                ps = ps_b.tile([128, ST], F32, tag="b")
                for c in range(KC):
                    nc.tensor.matmul(
                        ps, w_dkv[:, c, 128 * n : 128 * n + 128], xT[:, c, :],
                        start=(c == 0), stop=(c == KC - 1))
                nc.scalar.copy(out=ckvT[:, n, :], in_=ps)
            for n in range(2):
                ck(lambda n=n: ckv_chunk(n))

            def ckv_dma():
                ckv_out = bass.AP(tensor=ckvT_d, offset=s0,
                                  ap=[[S, 128], [128 * S, 2], [1, ST]])
                nc.gpsimd.dma_start(out=ckv_out, in_=ckvT)
            ck(ckv_dma)

            # roped projections -> qq/kk rows 64:128
            def rope_chunk(w_t, dst, n):
                ps = ps_b.tile([128, ST], F32, tag="b")
                for c in range(KC):
                    nc.tensor.matmul(
                        ps, w_t[:, c, 128 * n : 128 * n + 128], xT[:, c, :],
                        start=(c == 0), stop=(c == KC - 1))
                ta = ropet.tile([64, ST], F32, tag="ta")   # x1*cos
                tb = ropet.tile([64, ST], F32, tag="tb")   # x2*sin
                tc_ = ropet.tile([64, ST], F32, tag="tc")  # x1*sin
                td = ropet.tile([64, ST], F32, tag="td")   # x2*cos
                nc.vector.tensor_mul(ta, ps[0:64, :], cs[0:64, :])
                nc.vector.tensor_mul(tb, ps[64:128, :], cs[64:128, :])
                nc.vector.tensor_mul(tc_, ps[0:64, :], sc[0:64, :])
                nc.vector.tensor_mul(td, ps[64:128, :], sc[64:128, :])
                ha, hb = 2 * n, 2 * n + 1
                sl = slice(s0, s0 + ST)
                nc.gpsimd.tensor_sub(dst[64:96, ha, sl], ta[0:32, :], tb[0:32, :])
                nc.gpsimd.tensor_sub(dst[64:96, hb, sl], ta[32:64, :], tb[32:64, :])
                nc.gpsimd.tensor_add(dst[96:128, ha, sl], tc_[0:32, :], td[0:32, :])
                nc.gpsimd.tensor_add(dst[96:128, hb, sl], tc_[32:64, :], td[32:64, :])
            for w_t, dst in ((w_qp, qq), (w_kp, kk)):
                for n in range(2):
                    ck(lambda w_t=w_t, dst=dst, n=n: rope_chunk(w_t, dst, n))

            # k_c = W_uk^T @ c_kv^T -> kk rows 0:64
            def kc_chunk(n):
                ps = ps_b.tile([128, ST], F32, tag="b")
                for c in range(2):
                    nc.tensor.matmul(
                        ps, w_uk[:, c, 128 * n : 128 * n + 128], ckvT[:, c, :],
                        start=(c == 0), stop=(c == 1))
                nc.scalar.copy(out=kk[0:64, 2 * n, s0 : s0 + ST], in_=ps[0:64, :])
                nc.scalar.copy(out=kk[0:64, 2 * n + 1, s0 : s0 + ST], in_=ps[64:128, :])
            for n in range(2):
                ck(lambda n=n: kc_chunk(n))

            # v_c natural [sk, d]
            def v_chunk(sb):
                blk = st * (ST // 128) + sb
                pv = ps_b.tile([128, HL * DH], F32, tag="b")
                for c in range(2):
                    nc.tensor.matmul(
                        pv, ckvT[:, c, sb * 128 : (sb + 1) * 128], w_uv[:, c, :],
                        start=(c == 0), stop=(c == 1))
                for h in range(HL):
                    if h < 1:
                        nc.scalar.copy(
                            out=vv[:, blk, 65 * h : 65 * h + 64],
                            in_=pv[:, 64 * h : 64 * h + 64])
                    else:
                        nc.vector.tensor_copy(
                            out=vv[:, blk, 65 * h : 65 * h + 64],
                            in_=pv[:, 64 * h : 64 * h + 64])
            for sb in range(ST // 128):
                ck(lambda sb=sb: v_chunk(sb))

            # stream k_r out as each tile of kk rows 64:128 completes
            def kr_dma():
                for h in range(HL):
                    nc.gpsimd.dma_start(
                        out=krT_d[h, :, s0 : s0 + ST], in_=kk[64:128, h, s0 : s0 + ST])
            ck(kr_dma)
            return chunks

        def attn_chunks(jt):
            chunks = []
            ck = chunks.append
            q0 = jt * ST
            nblk = 4 * (jt + 1) if causal else NSB
            def do_norm(h, pc):
                # normalize: ctxT[:, h] = pc[0:64] / Z,  Z = pc[64]
                rr = rp.tile([1, ST], F32, tag="rr")
                with nc.allow_low_precision("fp32 reciprocal"):
                    nc.vector.reciprocal(rr, pc[64:65, :])
                rb = rp.tile([64, ST], F32, tag="rb")
                nc.gpsimd.partition_broadcast(rb, rr[0:1, :])
                nc.vector.tensor_mul(
                    ctxT[64 * (h % 2) : 64 * (h % 2) + 64, h // 2, q0 : q0 + ST],
                    pc[0:64, :], rb)

            LAG = 7
            state = {"pcs": {}, "pend": [], "norm_q": []}

            def emit_ctx(h, flush_last):
                i0, off0, w0, base = state["pend"].pop(0)
                pc = state["pcs"][h]
                nc.tensor.matmul(
                    pc[:, off0:ST], vv[:, i0, 65 * h : 65 * h + 65],
                    w0[:, base + off0 : base + ST],
                    start=(i0 == 0), stop=(flush_last and i0 == nblk - 1))

            def blk_chunk(h, i, two):
                # processes sk-blocks i (and i+1 when two=True, both full)
                if i == 0:
                    pc_new = ps_ctx.tile([65, ST], F32, tag="ctx")
                    state["pcs"][h] = pc_new
                off = max(0, 128 * i - q0) if causal else 0
                width = 2 * ST if two else ST
                ps = ps_a.tile([128, width], F32, tag="a")
                nc.tensor.matmul(
                    ps[:, off:ST], kk[:, h, 128 * i : 128 * i + 128],
                    qq[:, h, q0 + off : q0 + ST],
                    start=True, stop=True)
                if two:
                    # full width even in the diag region: the sq<sk columns are
                    # junk but finite, never consumed, and keep the psum fully
                    # written so one exp can span the pair
                    nc.tensor.matmul(
                        ps[:, ST : 2 * ST],
                        kk[:, h, 128 * (i + 1) : 128 * (i + 2)],
                        qq[:, h, q0 : q0 + ST],
                        start=True, stop=True)
                if use_mask:
                    mt = mp.tile([128, ST], F32, tag="mt")
                    nc.sync.dma_start(
                        out=mt, in_=maskT_d[128 * i : 128 * i + 128, q0 : q0 + ST])
                    nc.vector.tensor_add(ps[:, 0:ST], ps[:, 0:ST], mt)
                    if two:
                        mt2 = mp.tile([128, ST], F32, tag="mt")
                        nc.sync.dma_start(
                            out=mt2,
                            in_=maskT_d[128 * (i + 1) : 128 * (i + 2), q0 : q0 + ST])
                        nc.vector.tensor_add(ps[:, ST : 2 * ST], ps[:, ST : 2 * ST], mt2)
                off2 = (max(0, 128 * (i + 1) - q0) if causal else 0) if two else 0
                if causal and i >= 4 * jt:
                    nc.vector.tensor_add(
                        ps[:, off : off + 128], ps[:, off : off + 128], tri_sb)
                if two and causal and i + 1 >= 4 * jt:
                    nc.vector.tensor_add(
                        ps[:, ST + off2 : ST + off2 + 128],
                        ps[:, ST + off2 : ST + off2 + 128], tri_sb)
                w = wp.tile([128, width], F32R, tag="w")
                nc.scalar.activation(
                    out=w[:, off:width], in_=ps[:, off:width],
                    func=mybir.ActivationFunctionType.Exp)
                # i==0 always has off==0, so the first matmul of the
                # accumulation group covers the full [65, ST] region.
                state["pend"].append((i, off, w, 0))
                if two:
                    state["pend"].append((i + 1, off2, w, ST))
                while len(state["pend"]) > LAG:
                    emit_ctx(h, False)
                if 1 <= i <= 2 and state["norm_q"]:
                    do_norm(*state["norm_q"].pop())

            def head_flush(h):
                while state["pend"]:
                    emit_ctx(h, True)
                state["norm_q"].append((h, state["pcs"][h]))

            for h in range(HL):
                for i in range(0, nblk, 2):
                    ck(lambda h=h, i=i: blk_chunk(h, i, True))
                ck(lambda h=h: head_flush(h))
            ck(lambda: do_norm(*state["norm_q"].pop()))

            # out projection for this sq tile
            def out_chunk(sb):
                r0 = q0 + sb * 128
                ob = obp.tile([128, D], F32, tag="ob")
                for mtile in range(2):
                    po = ps_b.tile([128, ST], F32, tag="b")
                    for c in range(2):
                        nc.tensor.matmul(
                            po, ctxT[:, c, r0 : r0 + 128],
                            w_o[:, c, mtile * ST : (mtile + 1) * ST],
                            start=(c == 0), stop=(c == 1))
                    nc.vector.tensor_copy(
                        out=ob[:, mtile * ST : (mtile + 1) * ST], in_=po)
                nc.gpsimd.dma_start(out=outp_d[r0 : r0 + 128, :], in_=ob)
            for sb in range(ST // 128):
                ck(lambda sb=sb: out_chunk(sb))
            return chunks

        def run_interleaved(A, B):
            ia = ib = 0
            while ia < len(A) or ib < len(B):
                fa = ia / len(A) if A else 2.0
                fb = ib / len(B) if B else 2.0
                if ia < len(A) and (fa <= fb or ib >= len(B)):
                    A[ia]()
                    ia += 1
                else:
                    B[ib]()
                    ib += 1

        if causal:
            # fused: attention for sq-tile st needs only s-tiles <= st;
            # interleave attn(st) with proj(st+1) for engine overlap
            run_interleaved(proj_chunks(0), weight_chunks())
            for st in range(NST):
                A = attn_chunks(st)
                Bc = proj_chunks(st + 1) if st + 1 < NST else []
                run_interleaved(A, Bc)
        else:
            run_interleaved(proj_chunks(0), weight_chunks())
            for st in range(1, NST):
                for c in proj_chunks(st):
                    c()
            for jt in range(NST):
                for c in attn_chunks(jt):
                    c()

        if debug:
            nc.sync.dma_start(out=qq_dbg[:, :, :], in_=qq)
            nc.sync.dma_start(out=kk_dbg[:, :, :], in_=kk)
            nc.sync.dma_start(out=vv_dbg[:, :, :], in_=vv)
            nc.sync.dma_start(out=ctxT_dbg[:, :, :], in_=ctxT)

    nc.finalize()
    return nc


# ---------------------------------------------------------------------------
# host side
# ---------------------------------------------------------------------------

def _rope_tables_np(S):
    theta = 1.0 / (10000.0 ** (np.arange(0, DR, 2, dtype=np.float32) / DR))
    freqs = np.outer(np.arange(S, dtype=np.float32), theta)  # [S, 32]
    return np.cos(freqs).T.copy(), np.sin(freqs).T.copy()    # [32, S]


def _chunk(w, kc):
    """[kc*128, N] -> [128, kc, N] contiguous"""
    n = w.shape[1]
    return np.ascontiguousarray(
        w.reshape(kc, 128, n).transpose(1, 0, 2), dtype=np.float32
    )


def _perm_cols(g):
    """column order for W_{q,k}_pos slice of head group g (len 256)"""
    cols = []
    for n in range(2):
        ha, hb = 4 * g + 2 * n, 4 * g + 2 * n + 1
        cols += [64 * ha + 2 * i for i in range(32)]       # ha evens
        cols += [64 * hb + 2 * i for i in range(32)]       # hb evens
        cols += [64 * ha + 2 * i + 1 for i in range(32)]   # ha odds
        cols += [64 * hb + 2 * i + 1 for i in range(32)]   # hb odds
    return np.array(cols)


def prep_in_maps(x, attn_mask, W_q, W_dkv, W_uk, W_uv, W_k_pos, W_q_pos, W_o, S):
    KC = D // 128
    cosT, sinT = _rope_tables_np(S)
    cos2 = np.concatenate([cosT, cosT], 0)
    sin2 = np.concatenate([sinT, sinT], 0)

    m = np.asarray(attn_mask, np.float32).reshape(S, S)
    causal_ref = np.where(np.tril(np.ones((S, S), bool)), 0.0, -1e9).astype(np.float32)
    if np.array_equal(m, causal_ref):
        causal, use_mask = True, False
    elif not m.any():
        causal, use_mask = False, False
    else:
        causal, use_mask = False, True

    tri = np.where(
        np.arange(128)[:, None] <= np.arange(128)[None, :], 0.0, -1e9
    ).astype(np.float32)

    in_maps = []
    for core in range(NCORES):
        b, g = core // 4, core % 4
        sl = slice(256 * g, 256 * g + 256)
        pc = _perm_cols(g)
        im = {
            "xbT": np.ascontiguousarray(
                x[b].T.reshape(D // 128, 128, S), np.float32),
            "wq": _chunk(W_q[:, sl] * 0.125, KC),
            "wqp": _chunk(W_q_pos[:, pc] * 0.125, KC),
            "wkp": _chunk(W_k_pos[:, pc], KC),
            "wdkv": _chunk(W_dkv, KC),
            "wuk": _chunk(W_uk[:, sl], 2),
            "wuv": _chunk(W_uv[:, sl], 2),
            "wo": _chunk(W_o[sl, :], 2),
            "cos2": cos2,
            "sin2": sin2,
            "vones": np.ones((128, HL * 65), np.float32),
            "ones64": np.ones((1, 64), np.float32),
        }
        if causal:
            im["tri"] = tri
        if use_mask:
            im["maskT"] = np.ascontiguousarray(m.T)
        in_maps.append(im)
    return in_maps, causal, use_mask


def assemble(results, S):
    out = np.zeros((B, S, D), np.float32)
    c_kv = np.zeros((B, S, DL), np.float32)
    k_r = np.zeros((B, NH, S, DR), np.float32)
    for core in range(NCORES):
        b, g = core // 4, core % 4
        r = results[core]
        out[b] += r["outp"]
        if g == 0:
            c_kv[b] = r["ckvT"].reshape(DL, S).T
        krT = r["krT"]  # [HL, 64, S]
        for h in range(HL):
            k_r[b, 4 * g + h, :, 0::2] = krT[h, 0:32, :].T
            k_r[b, 4 * g + h, :, 1::2] = krT[h, 32:64, :].T
    return out, c_kv, k_r


_NC_CACHE = {}


def get_nc(S, causal, use_mask):
    key = (S, causal, use_mask)
    if key not in _NC_CACHE:
        _NC_CACHE[key] = build_nc(S, causal, use_mask)
    return _NC_CACHE[key]


def kernel(x, attn_mask, W_q, W_dkv, W_uk, W_uv, W_k_pos, W_q_pos, W_o,
           _trace=False, _trace_kwargs=None):
    from concourse.bass_utils import run_bass_kernel_spmd

    x = np.asarray(x, np.float32)
    S = x.shape[1]
    args = [np.asarray(a, np.float32) for a in
            (W_q, W_dkv, W_uk, W_uv, W_k_pos, W_q_pos, W_o)]
    in_maps, causal, use_mask = prep_in_maps(x, attn_mask, args[0], args[1],
                                             args[2], args[3], args[4], args[5],
                                             args[6], S)
    nc = get_nc(S, causal, use_mask)
    res = run_bass_kernel_spmd(
        nc, in_maps, list(range(NCORES)),
        trace=_trace, **(_trace_kwargs or {}),
    )
    out = assemble(res.results, S)
    if _trace:
        return out, res
    return out
